# revision 11
# baseline (speedup 1.0000x reference)
"""Longformer sliding-window attention on 8 trn2 NeuronCores.

B=2, H=12, L=4096, D=64, one-sided window w=256 (full window 513).
Shard: 24 (b,h) pairs -> 3 heads per core.

Per-core algorithm (per head, 32 key-blocks of 128 keys):
  S^T[kb] = (K_blk)^T-part scores: (128 keys x 640 queries) via 2 f32r matmuls
            (lhsT = K^T block (64,128), rhs = Q^T span (64,<=640))
  P^T = exp(S/8) on ScalarE (PSUM->SBUF, bf16), band edges masked
            multiplicatively on VectorE (two 128x128 triangle masks)
  ctx/denoms: PV matmul with V2 = [V | ones] (128 keys x 128):
            out rows 0:64 = unnormalized ctx^T, rows 64:128 = softmax
            denominator replicated 64x -- both accumulate in (128,512)
            PSUM tiles per 512-query chunk.
  normalize: ctx^T * reciprocal_approx_fast(denom) on VectorE -> SBUF -> DMA.

Output assembled host-side: (24, 64, 4096) -> (2,12,4096,64) transpose only.
"""

import sys

sys.path.insert(0, "/opt/trn_rl_repo")

import numpy as np
import ml_dtypes

B, H, L, D = 2, 12, 4096, 64
W = 256            # one-sided window
NCORES = 8
HPC = (B * H) // NCORES   # heads per core = 3
BLK = 128                 # key block (partition dim)
NB = L // BLK             # 32 key blocks per head
SPAN = 2 * W + BLK        # 640 query columns per key block
CTXW = 512                # ctx psum tile width (1 bank)
NT = L // CTXW            # 8 ctx tiles per head

_CACHE = {}
QK_DTYPE = "f32r"
DEBUG_OUT = "norm"   # norm | ctx | den
REPEAT = 1           # duplicate compute body for wall-clock timing


def _build_program():
    import concourse.bacc as bacc
    import concourse.mybir as mybir
    import concourse.tile as tile

    f32 = mybir.dt.float32
    f32r = mybir.dt.float32r
    bf16 = mybir.dt.bfloat16

    nc = bacc.Bacc("TRN2", target_bir_lowering=False, debug=False)

    qt_d = nc.dram_tensor("qt", [HPC, D, L], f32, kind="ExternalInput").ap()
    kt_d = nc.dram_tensor("kt", [HPC, D, L], f32, kind="ExternalInput").ap()
    v_d = nc.dram_tensor("v", [HPC, L, D], f32, kind="ExternalInput").ap()
    em_d = nc.dram_tensor("masks", [BLK, 2, BLK], bf16, kind="ExternalInput").ap()
    out_d = nc.dram_tensor("out", [HPC, D, L], f32, kind="ExternalOutput").ap()

    # per-(kb) geometry
    geo = []
    for kb in range(NB):
        K0 = BLK * kb
        qbase = K0 - W
        qlo = max(0, qbase)
        qhi = min(L, K0 + BLK + W)
        geo.append((K0, qbase, qlo, qhi))

    # ctx-tile contributors
    contrib = {t: [] for t in range(NT)}
    for kb, (K0, qbase, qlo, qhi) in enumerate(geo):
        for t in range(qlo // CTXW, (qhi - 1) // CTXW + 1):
            contrib[t].append(kb)
    last_kb = {t: kbs[-1] for t, kbs in contrib.items()}

    with tile.TileContext(nc) as tc:
        with (
            tc.tile_pool(name="const", bufs=1) as constp,
            tc.tile_pool(name="qk", bufs=2) as qkp,
            tc.tile_pool(name="vst", bufs=2) as vstp,
            tc.tile_pool(name="pt", bufs=3) as ptp,
            tc.tile_pool(name="outb", bufs=2) as outp,
            tc.tile_pool(name="rp", bufs=2) as rp,
            tc.tile_pool(name="st", bufs=2, space="PSUM") as stp,
            tc.tile_pool(name="ctx", bufs=4, space="PSUM") as ctxp,
        ):
            em = constp.tile([BLK, 2, BLK], bf16)
            nc.sync.dma_start(out=em, in_=em_d)

            v2 = constp.tile([BLK, NB, 2 * D], bf16)   # [ones | V] per block
            nc.vector.memset(v2[:, :, 0:D], 1.0)

            for h in [hh for _ in range(REPEAT) for hh in range(HPC)]:
                qt_s = qkp.tile([D, L], f32)
                nc.sync.dma_start(out=qt_s, in_=qt_d[h])
                kt_s = qkp.tile([D, L], f32)
                nc.sync.dma_start(out=kt_s, in_=kt_d[h])
                if QK_DTYPE == "f32r":
                    # f32r matmul operands need a rounding producer:
                    # GpSimd copy keeps VectorE free for softmax work
                    qt_t = qkp.tile([D, L], f32r)
                    nc.gpsimd.tensor_copy(qt_t, qt_s)
                    kt_t = qkp.tile([D, L], f32r)
                    nc.gpsimd.tensor_copy(kt_t, kt_s)
                else:
                    qt_t, kt_t = qt_s, kt_s

                vstg = vstp.tile([BLK, NB, D], f32)
                nc.sync.dma_start(
                    out=vstg, in_=v_d[h].rearrange("(nb p) d -> p nb d", p=BLK)
                )
                nc.vector.tensor_copy(v2[:, :, D:], vstg)  # f32 -> bf16 cast

                outbuf = outp.tile([2 * D, L], f32)
                ctx_tiles = {}
                ctx_started = set()

                for kb in range(NB):
                    K0, qbase, qlo, qhi = geo[kb]
                    c_lo, c_hi = qlo - qbase, qhi - qbase

                    st = stp.tile([BLK, SPAN], f32)
                    lhsT = kt_t[:, K0 : K0 + BLK]
                    pieces = []
                    if c_lo < 512:
                        pieces.append((c_lo, min(512, c_hi)))
                    if c_hi > 512:
                        pieces.append((512, c_hi))
                    for (a, b) in pieces:
                        nc.tensor.matmul(
                            st[:, a:b],
                            lhsT,
                            qt_t[:, qbase + a : qbase + b],
                            start=True,
                            stop=True,
                        )

                    pt = ptp.tile([BLK, SPAN], bf16)
                    nc.scalar.activation(
                        pt[:, c_lo:c_hi],
                        st[:, c_lo:c_hi],
                        mybir.ActivationFunctionType.Exp,
                        scale=float(1.0 / np.sqrt(D)),
                    )

                    ptr = pt.rearrange("p (s c) -> p s c", c=BLK)
                    if c_lo == 0:   # left triangle mask applies (cols 0:128 valid)
                        nc.vector.tensor_mul(ptr[:, 0, :], ptr[:, 0, :], em[:, 0, :])
                    if c_hi == SPAN:  # right triangle mask (cols 512:640 valid)
                        nc.vector.tensor_mul(ptr[:, 4, :], ptr[:, 4, :], em[:, 1, :])

                    for t in range(qlo // CTXW, (qhi - 1) // CTXW + 1):
                        a = max(qlo, CTXW * t)
                        b = min(qhi, CTXW * (t + 1))
                        if t not in ctx_tiles:
                            ctx_tiles[t] = ctxp.tile(
                                [BLK, CTXW], f32, name="ctx_t", tag="ctx_t"
                            )
                        first = t not in ctx_started
                        ctx_started.add(t)
                        nc.tensor.matmul(
                            ctx_tiles[t][:, a - CTXW * t : b - CTXW * t],
                            v2[:, kb, :],
                            pt[:, a - qbase : b - qbase],
                            start=first,
                            stop=(kb == last_kb[t]),
                        )

                    for t in list(ctx_tiles):
                        if last_kb[t] == kb:
                            ct = ctx_tiles.pop(t)
                            ob = outbuf[D : 2 * D, CTXW * t : CTXW * (t + 1)]
                            if DEBUG_OUT == "ctx":
                                nc.vector.tensor_copy(ob, ct[D : 2 * D, :])
                            elif DEBUG_OUT == "den":
                                nc.vector.tensor_copy(ob, ct[0:D, :])
                            else:
                                # denoms on partitions 0:64 (custom-DVE recip
                                # needs base partition 0); DMA-shift recip up
                                # to 64:128 where the ctx rows live, mul there
                                rlo = rp.tile([D, CTXW], f32, name="rlo")
                                nc.vector.reciprocal_approx_fast(
                                    out=rlo, in_=ct[0:D, :]
                                )
                                rhi = rp.tile([2 * D, CTXW], f32, name="rhi")
                                nc.sync.dma_start(out=rhi[D : 2 * D, :], in_=rlo)
                                nc.vector.tensor_mul(
                                    ob, ct[D : 2 * D, :], rhi[D : 2 * D, :]
                                )

                nc.sync.dma_start(out=out_d[h], in_=outbuf[D : 2 * D, :])

    nc.compile()
    return nc


def _get_nc():
    if "nc" not in _CACHE:
        _CACHE["nc"] = _build_program()
    return _CACHE["nc"]


def _host_prep(q, k, v):
    qf = np.ascontiguousarray(
        np.asarray(q, dtype=np.float32).transpose(0, 1, 3, 2)
    ).reshape(B * H, D, L)
    kf = np.ascontiguousarray(
        np.asarray(k, dtype=np.float32).transpose(0, 1, 3, 2)
    ).reshape(B * H, D, L)
    vf = np.ascontiguousarray(np.asarray(v, dtype=np.float32)).reshape(B * H, L, D)

    i = np.arange(BLK)
    em = np.zeros((BLK, 2, BLK), dtype=ml_dtypes.bfloat16)
    em[:, 0, :] = (i[None, :] >= i[:, None]).astype(ml_dtypes.bfloat16)  # left: col>=row
    em[:, 1, :] = (i[None, :] <= i[:, None]).astype(ml_dtypes.bfloat16)  # right: col<=row

    in_maps = []
    for c in range(NCORES):
        sl = slice(c * HPC, (c + 1) * HPC)
        in_maps.append(
            {
                "qt": np.ascontiguousarray(qf[sl]),
                "kt": np.ascontiguousarray(kf[sl]),
                "v": np.ascontiguousarray(vf[sl]),
                "masks": em,
            }
        )
    return in_maps


def kernel(q, k, v, padding_mask):
    from concourse.bass_utils import run_bass_kernel_spmd

    pm = np.asarray(padding_mask)
    assert pm.all(), "kernel specialized for all-ones padding mask"

    nc = _get_nc()
    in_maps = _host_prep(q, k, v)
    res = run_bass_kernel_spmd(nc, in_maps, core_ids=list(range(NCORES)))
    outs = [res.results[c]["out"] for c in range(NCORES)]  # each (HPC, 64, 4096)
    full = np.concatenate(outs, axis=0)                     # (24, 64, 4096)
    ctx = full.transpose(0, 2, 1).reshape(B, H, L, D)
    return np.ascontiguousarray(ctx.astype(np.float32))


# revision 17
# speedup vs baseline: 1.4230x; 1.4230x over previous
"""Longformer sliding-window attention on 8 trn2 NeuronCores.

B=2, H=12, L=4096, D=64, one-sided window w=256 (full window 513).
Shard: 24 (b,h) pairs -> 3 heads per core.

Per-core algorithm (per head, 32 key-blocks of 128 keys):
  S^T[kb] = (K_blk)^T-part scores: (128 keys x 640 queries) via 2 f32r matmuls
            (lhsT = K^T block (64,128), rhs = Q^T span (64,<=640))
  P^T = exp(S/8) on ScalarE (PSUM->SBUF, bf16), band edges masked
            multiplicatively on VectorE (two 128x128 triangle masks)
  ctx/denoms: PV matmul with V2 = [V | ones] (128 keys x 128):
            out rows 0:64 = unnormalized ctx^T, rows 64:128 = softmax
            denominator replicated 64x -- both accumulate in (128,512)
            PSUM tiles per 512-query chunk.
  normalize: ctx^T * reciprocal_approx_fast(denom) on VectorE -> SBUF -> DMA.

Output assembled host-side: (24, 64, 4096) -> (2,12,4096,64) transpose only.
"""

import sys

sys.path.insert(0, "/opt/trn_rl_repo")

import numpy as np
import ml_dtypes

B, H, L, D = 2, 12, 4096, 64
W = 256            # one-sided window
NCORES = 8
HPC = (B * H) // NCORES   # heads per core = 3
BLK = 128                 # key block (partition dim)
NB = L // BLK             # 32 key blocks per head
SPAN = 2 * W + BLK        # 640 query columns per key block
CTXW = 512                # ctx psum tile width (1 bank)
NT = L // CTXW            # 8 ctx tiles per head

_CACHE = {}
QK_DTYPE = "f32r"
DEBUG_OUT = "norm"   # norm | ctx | den
REPEAT = 1           # duplicate compute body for wall-clock timing
SKIP_MASKS = False


def _build_program():
    import concourse.bacc as bacc
    import concourse.bass as bass
    import concourse.mybir as mybir
    import concourse.tile as tile

    f32 = mybir.dt.float32
    f32r = mybir.dt.float32r
    bf16 = mybir.dt.bfloat16

    nc = bacc.Bacc("TRN2", target_bir_lowering=False, debug=False)

    qt_d = nc.dram_tensor("qt", [HPC, D, L], f32r, kind="ExternalInput").ap()
    kt_d = nc.dram_tensor("kt", [HPC, D, L], f32r, kind="ExternalInput").ap()
    qtb_d = nc.dram_tensor("qtb", [HPC, D, L], bf16, kind="ExternalInput").ap()
    ktb_d = nc.dram_tensor("ktb", [HPC, D, L], bf16, kind="ExternalInput").ap()
    v_d = nc.dram_tensor("v", [HPC, BLK, NB, D], f32, kind="ExternalInput").ap()
    em_d = nc.dram_tensor("masks", [BLK, 2, BLK], bf16, kind="ExternalInput").ap()
    out_d = nc.dram_tensor("out", [HPC, D, L], f32, kind="ExternalOutput").ap()

    # per-(kb) geometry
    geo = []
    for kb in range(NB):
        K0 = BLK * kb
        qbase = K0 - W
        qlo = max(0, qbase)
        qhi = min(L, K0 + BLK + W)
        geo.append((K0, qbase, qlo, qhi))

    # ctx-tile contributors
    contrib = {t: [] for t in range(NT)}
    for kb, (K0, qbase, qlo, qhi) in enumerate(geo):
        for t in range(qlo // CTXW, (qhi - 1) // CTXW + 1):
            contrib[t].append(kb)
    last_kb = {t: kbs[-1] for t, kbs in contrib.items()}

    with tile.TileContext(nc) as tc:
        with (
            tc.tile_pool(name="const", bufs=1) as constp,
            tc.tile_pool(name="qk", bufs=2) as qkp,
            tc.tile_pool(name="vst", bufs=2) as vstp,
            tc.tile_pool(name="pt", bufs=3) as ptp,
            tc.tile_pool(name="outb", bufs=2) as outp,
            tc.tile_pool(name="rp", bufs=2) as rp,
            tc.tile_pool(name="st", bufs=2, space="PSUM") as stp,
            tc.tile_pool(name="ctx", bufs=4, space="PSUM") as ctxp,
        ):
            em = constp.tile([BLK, 2, BLK], bf16)
            nc.sync.dma_start(out=em, in_=em_d)

            v2 = constp.tile([BLK, NB, 2 * D], bf16)   # [ones | V] per block
            nc.vector.memset(v2[:, :, 0:D], 1.0)

            for h in [hh for _ in range(REPEAT) for hh in range(HPC)]:
                qt_t = qkp.tile([D, L], f32r)
                nc.sync.dma_start(out=qt_t, in_=qt_d[h])
                kt_t = qkp.tile([D, L], f32r)
                nc.sync.dma_start(out=kt_t, in_=kt_d[h])
                qtb_t = qkp.tile([D, L], bf16)
                nc.sync.dma_start(out=qtb_t, in_=qtb_d[h])
                ktb_t = qkp.tile([D, L], bf16)
                nc.sync.dma_start(out=ktb_t, in_=ktb_d[h])

                vstg = vstp.tile([BLK, NB, D], f32)
                nc.sync.dma_start(out=vstg, in_=v_d[h])
                nc.gpsimd.tensor_copy(v2[:, :, D:], vstg)  # f32 -> bf16 cast off DVE

                outbuf = outp.tile([2 * D, L], f32)
                ctx_tiles = {}
                ctx_started = set()

                for kb in range(NB):
                    K0, qbase, qlo, qhi = geo[kb]
                    c_lo, c_hi = qlo - qbase, qhi - qbase

                    st = stp.tile([BLK, SPAN], f32)
                    if c_lo < 512:
                        a, b = c_lo, min(512, c_hi)
                        nc.tensor.matmul(
                            st[:, a:b],
                            kt_t[:, K0 : K0 + BLK],
                            qt_t[:, qbase + a : qbase + b],
                            start=True,
                            stop=True,
                        )
                    if c_hi > 512:
                        # 128-wide tail in bf16: f32r pays 4 cyc/row below N=256
                        nc.tensor.matmul(
                            st[:, 512:c_hi],
                            ktb_t[:, K0 : K0 + BLK],
                            qtb_t[:, qbase + 512 : qbase + c_hi],
                            start=True,
                            stop=True,
                        )

                    pt = ptp.tile([BLK, SPAN], bf16)
                    nc.scalar.activation(
                        pt[:, c_lo:c_hi],
                        st[:, c_lo:c_hi],
                        mybir.ActivationFunctionType.Exp,
                        scale=float(1.0 / np.sqrt(D)),
                    )

                    ptr = pt.rearrange("p (s c) -> p s c", c=BLK)
                    if SKIP_MASKS:
                        pass
                    elif c_lo == 0 and c_hi == SPAN:
                        # both triangle masks in one strided op (cols 0:128 + 512:640)
                        pte = bass.AP(
                            tensor=pt.tensor,
                            offset=pt.offset,
                            ap=[pt.ap[0], [4 * BLK, 2], [1, BLK]],
                        )
                        nc.vector.tensor_mul(pte, pte, em)
                    elif c_lo == 0:
                        nc.vector.tensor_mul(ptr[:, 0, :], ptr[:, 0, :], em[:, 0, :])
                    elif c_hi == SPAN:
                        nc.vector.tensor_mul(ptr[:, 4, :], ptr[:, 4, :], em[:, 1, :])

                    for t in range(qlo // CTXW, (qhi - 1) // CTXW + 1):
                        a = max(qlo, CTXW * t)
                        b = min(qhi, CTXW * (t + 1))
                        if t not in ctx_tiles:
                            ctx_tiles[t] = ctxp.tile(
                                [BLK, CTXW], f32, name="ctx_t", tag="ctx_t"
                            )
                        first = t not in ctx_started
                        ctx_started.add(t)
                        nc.tensor.matmul(
                            ctx_tiles[t][:, a - CTXW * t : b - CTXW * t],
                            v2[:, kb, :],
                            pt[:, a - qbase : b - qbase],
                            start=first,
                            stop=(kb == last_kb[t]),
                        )

                    for t in list(ctx_tiles):
                        if last_kb[t] == kb:
                            ct = ctx_tiles.pop(t)
                            ob = outbuf[D : 2 * D, CTXW * t : CTXW * (t + 1)]
                            if DEBUG_OUT == "ctx":
                                nc.vector.tensor_copy(ob, ct[D : 2 * D, :])
                            elif DEBUG_OUT == "den":
                                nc.vector.tensor_copy(ob, ct[0:D, :])
                            else:
                                # denoms on partitions 0:64 (custom-DVE recip
                                # needs base partition 0); DMA-shift recip up
                                # to 64:128 where the ctx rows live, mul there
                                rlo = rp.tile([D, CTXW], f32, name="rlo")
                                nc.vector.reciprocal_approx_fast(
                                    out=rlo, in_=ct[0:D, :]
                                )
                                rhi = rp.tile([2 * D, CTXW], f32, name="rhi")
                                nc.sync.dma_start(out=rhi[D : 2 * D, :], in_=rlo)
                                nc.vector.tensor_mul(
                                    ob, ct[D : 2 * D, :], rhi[D : 2 * D, :]
                                )

                nc.sync.dma_start(out=out_d[h], in_=outbuf[D : 2 * D, :])

    nc.compile()
    return nc


def _get_nc():
    if "nc" not in _CACHE:
        _CACHE["nc"] = _build_program()
    return _CACHE["nc"]


def _host_prep(q, k, v):
    qf = np.ascontiguousarray(
        np.asarray(q, dtype=np.float32).transpose(0, 1, 3, 2)
    ).reshape(B * H, D, L)
    kf = np.ascontiguousarray(
        np.asarray(k, dtype=np.float32).transpose(0, 1, 3, 2)
    ).reshape(B * H, D, L)
    vf = np.ascontiguousarray(
        np.asarray(v, dtype=np.float32)
        .reshape(B * H, NB, BLK, D)
        .transpose(0, 2, 1, 3)
    )

    qb16 = qf.astype(ml_dtypes.bfloat16)
    kb16 = kf.astype(ml_dtypes.bfloat16)

    i = np.arange(BLK)
    em = np.zeros((BLK, 2, BLK), dtype=ml_dtypes.bfloat16)
    em[:, 0, :] = (i[None, :] >= i[:, None]).astype(ml_dtypes.bfloat16)  # left: col>=row
    em[:, 1, :] = (i[None, :] <= i[:, None]).astype(ml_dtypes.bfloat16)  # right: col<=row

    in_maps = []
    for c in range(NCORES):
        sl = slice(c * HPC, (c + 1) * HPC)
        in_maps.append(
            {
                "qt": np.ascontiguousarray(qf[sl]),
                "kt": np.ascontiguousarray(kf[sl]),
                "qtb": np.ascontiguousarray(qb16[sl]),
                "ktb": np.ascontiguousarray(kb16[sl]),
                "v": np.ascontiguousarray(vf[sl]),
                "masks": em,
            }
        )
    return in_maps


def kernel(q, k, v, padding_mask):
    from concourse.bass_utils import run_bass_kernel_spmd

    pm = np.asarray(padding_mask)
    assert pm.all(), "kernel specialized for all-ones padding mask"

    nc = _get_nc()
    in_maps = _host_prep(q, k, v)
    res = run_bass_kernel_spmd(nc, in_maps, core_ids=list(range(NCORES)))
    outs = [res.results[c]["out"] for c in range(NCORES)]  # each (HPC, 64, 4096)
    full = np.concatenate(outs, axis=0)                     # (24, 64, 4096)
    ctx = full.transpose(0, 2, 1).reshape(B, H, L, D)
    return np.ascontiguousarray(ctx.astype(np.float32))
